# revision 13
# baseline (speedup 1.0000x reference)
"""CRF forward (log partition) Trainium2 kernel.

Algorithm (per core, data-parallel over batch):
  Keep the forward variable in exp space, laid out [T=48 partitions, B=32 free].
  Step s:  t_s = (E @ t_{s-1}) * e_s * 2^-SCALE_SHIFT      (E = exp(transitions))
  where the transition mix is a PE matmul with stationary lhsT = exp(transitions).T
  augmented with a ones column (row 48 of the PSUM result = sum_p t_{s-1}[p,b],
  used for periodic per-batch renormalization).  The emit factor e_s =
  exp(features[b, s-1, :]) is computed on the ACT engine in bulk chunks.
  Every NORM_K steps the state is renormalized by r = reciprocal(sum) (DVE
  reciprocal -> Pool partition_broadcast -> Pool multiply); each applied r is
  archived so the host can reconstruct the log scale exactly.
  All 2048 states are dumped to DRAM; the host gathers state at seq_len[b],
  adds the archived log scales, and finishes with the tiny terminal logsumexp.
"""

import sys

for _p in ("/opt/trn_rl_repo", "/root/.axon_site/_ro/trn_rl_repo"):
    if _p not in sys.path:
        sys.path.insert(0, _p)

import numpy as np

# ---- problem constants (hardcoded per contest contract) ----
B = 256
S = 2048
T = 48
START_TAG = 46
END_TAG = 47
N_CORES = 8
BL = B // N_CORES  # 32 batch elements per core

M_COLS = 65  # 48 mix cols + 16 dead cols + ones col at 64 (sum row, partition-aligned)
SUM_ROW = 64
SCALE_SHIFT = 6  # constant 2^-6 rescale per step
NORM_K = 32  # per-batch renorm every NORM_K steps
NORM_LAG = 3  # renorm factor is multiplied into e of step s_event + NORM_LAG
N_EVENTS = S // NORM_K  # 64
CHUNK_SP = 64  # s-pairs per feature chunk (=> 128 steps per chunk)
N_CHUNKS = S // (2 * CHUNK_SP)  # 16
N_DUMPS = S // 16  # dump 16 steps at a time
STATE_SLOTS = 32

_CACHED = {}
TRACE = False
LAST_RESULT = None


def _patch_ldw_opt():
    # the constant stationary operand makes per-matmul LDWEIGHTS redundant;
    # walrus elides them when ldw-opt is on
    import concourse.bass_utils as bu

    if getattr(bu, "_ldw_patched", False):
        return
    orig = bu.run_command

    def patched(argv, **kw):
        return orig(argv, **kw)

    bu.run_command = patched
    bu._ldw_patched = True


def _build_nc():
    import concourse.bacc as bacc
    import concourse.tile as tile
    from concourse import mybir

    _patch_ldw_opt()

    fp32 = mybir.dt.float32
    bf16 = mybir.dt.bfloat16

    nc = bacc.Bacc(
        "TRN2", target_bir_lowering=False, debug=False, num_devices=N_CORES
    )

    # I/O -----------------------------------------------------------------
    # features transposed on host to [96, S//2, BL]: partition p = (s%2)*48+t
    feats = nc.dram_tensor("feats_t", [128, S // 2, BL], fp32, kind="ExternalInput")
    # transitions.T augmented with a zeros column; device exps it (exp(0)=1).
    trans_aug = nc.dram_tensor("trans_aug", [T, M_COLS], fp32, kind="ExternalInput")
    init_state = nc.dram_tensor("init_state", [T, BL], fp32, kind="ExternalInput")

    dump = nc.dram_tensor("state_dump", [N_DUMPS, T, 16, BL], bf16, kind="ExternalOutput")
    r_out = nc.dram_tensor("r_out", [N_EVENTS, BL], fp32, kind="ExternalOutput")

    with tile.TileContext(nc) as tc:
        with (
            tc.tile_pool(name="singles", bufs=1) as singles,
            tc.tile_pool(name="feats", bufs=3) as fpool,
            tc.tile_pool(name="psum", bufs=2, space="PSUM") as ppool,
        ):
            # constants ---------------------------------------------------
            lhs_raw = singles.tile([T, M_COLS], fp32)
            nc.sync.dma_start(out=lhs_raw[:], in_=trans_aug[:])
            lhs_exp = singles.tile([T, M_COLS], bf16)
            nc.scalar.activation(lhs_exp[:], lhs_raw[:], mybir.ActivationFunctionType.Exp)
            # funnel const deps through DVE so matmul #1 waits on one engine only
            lhs_sb = singles.tile([T, M_COLS], bf16)
            nc.vector.tensor_copy(lhs_sb[:], lhs_exp[:])

            state = singles.tile([T, STATE_SLOTS, BL], bf16)
            init_raw = singles.tile([T, BL], fp32)
            nc.sync.dma_start(out=init_raw[:], in_=init_state[:])
            init_sb = singles.tile([T, BL], bf16)
            nc.vector.tensor_copy(init_sb[:], init_raw[:])

            r_buf = singles.tile([1, N_EVENTS, BL], fp32)
            rb = singles.tile([T, BL], fp32)
            ebias = singles.tile([128, 1], fp32)
            nc.vector.memset(ebias[:], -float(SCALE_SHIFT * np.log(2.0)))

            # psum: two tensors (alternate banks between consecutive steps)
            psA = ppool.tile([M_COLS, 8, BL], fp32, tag="psA")
            psB = ppool.tile([M_COLS, 8, BL], fp32, tag="psB")

            feat_tiles = []

            def load_chunk(c):
                # exp(x - SCALE_SHIFT*ln2): fold the per-step 2^-c rescale
                # into the emit factor so the step multiply is a plain TT op
                ftr = fpool.tile([128, CHUNK_SP, BL], fp32, tag="ftr")
                nc.sync.dma_start(
                    out=ftr[:], in_=feats[:, c * CHUNK_SP : (c + 1) * CHUNK_SP, :]
                )
                ft = fpool.tile([128, CHUNK_SP, BL], bf16, tag="ft")
                nc.scalar.activation(
                    ft[:], ftr[:], mybir.ActivationFunctionType.Exp, bias=ebias[:]
                )
                feat_tiles.append(ft)

            load_chunk(0)
            load_chunk(1)

            for s in range(1, S + 1):
                f_idx = s - 1
                s2, s_lo = divmod(f_idx, 2)
                c, s2c = divmod(s2, CHUNK_SP)

                # prefetch next chunk at the start of each chunk
                if s2c == 0 and s_lo == 0 and c >= 1 and c + 1 < N_CHUNKS:
                    load_chunk(c + 1)

                ft = feat_tiles[c]
                e_s = ft[64 * s_lo : 64 * s_lo + T, s2c, :]

                ps = (psA, psB)[s % 2]
                pslot = (s // 2) % 8
                rhs = (
                    init_sb[:]
                    if s == 1
                    else state[:, (s - 2) % STATE_SLOTS, :]
                )
                nc.tensor.matmul(
                    ps[:, pslot, :], lhs_sb[:], rhs, start=True, stop=True
                )

                dst = state[:, (s - 1) % STATE_SLOTS, :]
                nc.vector.tensor_mul(dst, ps[:T, pslot, :], e_s)

                if s % NORM_K == 0 and s + NORM_LAG <= S:
                    j = s // NORM_K - 1
                    nc.vector.reciprocal_approx_fast(
                        r_buf[:, j, :], ps[SUM_ROW : SUM_ROW + 1, pslot, :]
                    )
                    nc.gpsimd.partition_broadcast(rb[:], r_buf[:, j, :], channels=T)
                    fa_idx = s + NORM_LAG - 1
                    fa2, fa_lo = divmod(fa_idx, 2)
                    fac, fa2c = divmod(fa2, CHUNK_SP)
                    e_app = feat_tiles[fac][64 * fa_lo : 64 * fa_lo + T, fa2c, :]
                    nc.gpsimd.tensor_mul(e_app, e_app, rb[:])

                # dump every 16 steps
                if s % 16 == 0:
                    dc = s // 16 - 1
                    half = (dc % 2) * 16
                    nc.sync.dma_start(
                        out=dump[dc],
                        in_=state[:, half : half + 16, :],
                    )

            nc.sync.dma_start(out=r_out[:].unsqueeze(0), in_=r_buf[0:1, :, :])

    nc.finalize()
    return nc


def _get_nc():
    if "nc" not in _CACHED:
        _CACHED["nc"] = _build_nc()
    return _CACHED["nc"]


def kernel(features, transitions, seq_len):
    from concourse.bass_utils import run_bass_kernel_spmd

    features = np.asarray(features, dtype=np.float32)
    transitions = np.asarray(transitions, dtype=np.float32)
    seq_len_np = np.asarray(seq_len).astype(np.int64)

    nc = _get_nc()

    # host-side layout prep (sharding + transpose; no FLOPs beyond pad/transpose)
    trans_aug = np.full((T, M_COLS), -60000.0, dtype=np.float32)
    trans_aug[:, :T] = transitions.T  # lhsT[p, n] = transitions[n, p]
    trans_aug[:, SUM_ROW] = 0.0  # exp(0) = 1 -> ones column (sum row)
    init = np.zeros((T, BL), dtype=np.float32)
    init[START_TAG, :] = 1.0

    in_maps = []
    for c in range(N_CORES):
        fc = features[c * BL : (c + 1) * BL]  # [BL, S, T]
        # -> [128 = (s%2)*64+t (rows 48:64,112:128 pad), S//2, BL]
        ft = np.zeros((2, 64, S // 2, BL), dtype=np.float32)
        ft[:, :T] = fc.reshape(BL, S // 2, 2, T).transpose(2, 3, 1, 0)
        ft = np.ascontiguousarray(ft.reshape(128, S // 2, BL))
        in_maps.append(
            {"feats_t": ft, "trans_aug": trans_aug, "init_state": init}
        )

    global LAST_RESULT
    res = run_bass_kernel_spmd(
        nc, in_maps, core_ids=list(range(N_CORES)), trace=TRACE
    )
    LAST_RESULT = res

    # host epilogue ------------------------------------------------------
    ln2 = float(np.log(2.0))
    w_end = np.exp(transitions[END_TAG].astype(np.float64))  # [T]
    out = np.zeros(B, dtype=np.float64)
    for c in range(N_CORES):
        dump = res.results[c]["state_dump"].astype(np.float64)  # [128, T, 16, BL]
        r_arch = res.results[c]["r_out"].astype(np.float64)  # [128, BL]
        # log-scale applied up to and including step s:
        #   M_s = s * SCALE_SHIFT * ln2 - sum_{j: NORM_K*j <= s} ln r_j
        logr = np.log(r_arch)  # [N_EVENTS, BL]
        cum = np.cumsum(logr, axis=0)  # cum[j] = sum_{0..j}
        for bl in range(BL):
            L = int(seq_len_np[c * BL + bl])
            t_L = dump[(L - 1) // 16, :, (L - 1) % 16, bl]  # [T]
            M = L * SCALE_SHIFT * ln2
            # event j (1-based) applies its factor starting at step
            # NORM_K*j + NORM_LAG, i.e. included iff NORM_K*j + NORM_LAG <= L
            n_ev = (L - NORM_LAG) // NORM_K if L >= NORM_K + NORM_LAG else 0
            if n_ev > 0:
                M -= cum[n_ev - 1, bl]
            out[c * BL + bl] = np.log(np.dot(w_end, t_L)) + M
    return out.astype(np.float32)


# revision 14
# speedup vs baseline: 1.8141x; 1.8141x over previous
"""CRF forward (log partition) Trainium2 kernel.

Algorithm (per core, data-parallel over batch):
  Keep the forward variable in exp space, laid out [T=48 partitions, B=32 free].
  Step s:  t_s = (E @ t_{s-1}) * e_s * 2^-SCALE_SHIFT      (E = exp(transitions))
  where the transition mix is a PE matmul with stationary lhsT = exp(transitions).T
  augmented with a ones column (row 48 of the PSUM result = sum_p t_{s-1}[p,b],
  used for periodic per-batch renormalization).  The emit factor e_s =
  exp(features[b, s-1, :]) is computed on the ACT engine in bulk chunks.
  Every NORM_K steps the state is renormalized by r = reciprocal(sum) (DVE
  reciprocal -> Pool partition_broadcast -> Pool multiply); each applied r is
  archived so the host can reconstruct the log scale exactly.
  All 2048 states are dumped to DRAM; the host gathers state at seq_len[b],
  adds the archived log scales, and finishes with the tiny terminal logsumexp.
"""

import sys

for _p in ("/opt/trn_rl_repo", "/root/.axon_site/_ro/trn_rl_repo"):
    if _p not in sys.path:
        sys.path.insert(0, _p)

import numpy as np

# ---- problem constants (hardcoded per contest contract) ----
B = 256
S = 2048
T = 48
START_TAG = 46
END_TAG = 47
N_CORES = 8
BL = B // N_CORES  # 32 batch elements per core

M_COLS = 65  # 48 mix cols + 16 dead cols + ones col at 64 (sum row, partition-aligned)
SUM_ROW = 64
SCALE_SHIFT = 6  # constant 2^-6 rescale per step
NORM_K = 32  # per-batch renorm every NORM_K steps
NORM_LAG = 3  # renorm factor is multiplied into e of step s_event + NORM_LAG
N_EVENTS = S // NORM_K  # 64
CHUNK_SP = 64  # s-pairs per feature chunk (=> 128 steps per chunk)
N_CHUNKS = S // (2 * CHUNK_SP)  # 16
N_DUMPS = S // 16  # dump 16 steps at a time
STATE_SLOTS = 32

_CACHED = {}
TRACE = False
LAST_RESULT = None


def _patch_ldw_opt():
    # the constant stationary operand makes per-matmul LDWEIGHTS redundant;
    # walrus elides them when ldw-opt is on
    import concourse.bass_utils as bu

    if getattr(bu, "_ldw_patched", False):
        return
    orig = bu.run_command

    def patched(argv, **kw):
        return orig(argv, **kw)

    bu.run_command = patched
    bu._ldw_patched = True


def _build_nc():
    import concourse.bacc as bacc
    import concourse.tile as tile
    from concourse import mybir

    _patch_ldw_opt()

    fp32 = mybir.dt.float32
    bf16 = mybir.dt.bfloat16

    nc = bacc.Bacc(
        "TRN2", target_bir_lowering=False, debug=False, num_devices=N_CORES
    )

    # I/O -----------------------------------------------------------------
    # features transposed on host to [96, S//2, BL]: partition p = (s%2)*48+t
    feats = nc.dram_tensor("feats_t", [128, S // 2, BL], fp32, kind="ExternalInput")
    # transitions.T augmented with a zeros column; device exps it (exp(0)=1).
    trans_aug = nc.dram_tensor("trans_aug", [T, M_COLS], fp32, kind="ExternalInput")
    init_state = nc.dram_tensor("init_state", [T, BL], fp32, kind="ExternalInput")

    dump = nc.dram_tensor("state_dump", [N_DUMPS, T, 16, BL], bf16, kind="ExternalOutput")
    r_out = nc.dram_tensor("r_out", [N_EVENTS, BL], fp32, kind="ExternalOutput")

    with tile.TileContext(nc) as tc:
        with (
            tc.tile_pool(name="singles", bufs=1) as singles,
            tc.tile_pool(name="feats", bufs=3) as fpool,
            tc.tile_pool(name="psum", bufs=2, space="PSUM") as ppool,
        ):
            # constants ---------------------------------------------------
            lhs_raw = singles.tile([T, M_COLS], fp32)
            nc.sync.dma_start(out=lhs_raw[:], in_=trans_aug[:])
            lhs_exp = singles.tile([T, M_COLS], bf16)
            nc.scalar.activation(lhs_exp[:], lhs_raw[:], mybir.ActivationFunctionType.Exp)
            # funnel const deps through DVE so matmul #1 waits on one engine only
            lhs_sb = singles.tile([T, M_COLS], bf16)
            nc.vector.tensor_copy(lhs_sb[:], lhs_exp[:])

            state = singles.tile([T, STATE_SLOTS, BL], bf16)
            init_raw = singles.tile([T, BL], fp32)
            nc.sync.dma_start(out=init_raw[:], in_=init_state[:])
            init_sb = singles.tile([T, BL], bf16)
            nc.vector.tensor_copy(init_sb[:], init_raw[:])

            r_buf = singles.tile([1, N_EVENTS, BL], fp32)
            rb = singles.tile([T, BL], fp32)
            ebias = singles.tile([128, 1], fp32)
            nc.vector.memset(ebias[:], -float(SCALE_SHIFT * np.log(2.0)))

            # psum: two tensors (alternate banks between consecutive steps)
            psA = ppool.tile([M_COLS, 8, BL], fp32, tag="psA")
            psB = ppool.tile([M_COLS, 8, BL], fp32, tag="psB")

            feat_tiles = []

            def load_chunk(c):
                # exp(x - SCALE_SHIFT*ln2): fold the per-step 2^-c rescale
                # into the emit factor so the step multiply is a plain TT op
                ftr = fpool.tile([128, CHUNK_SP, BL], fp32, tag="ftr")
                nc.sync.dma_start(
                    out=ftr[:], in_=feats[:, c * CHUNK_SP : (c + 1) * CHUNK_SP, :]
                )
                ft = fpool.tile([128, CHUNK_SP, BL], bf16, tag="ft")
                nc.scalar.activation(
                    ft[:], ftr[:], mybir.ActivationFunctionType.Exp, bias=ebias[:]
                )
                feat_tiles.append(ft)

            load_chunk(0)
            load_chunk(1)

            for s in range(1, S + 1):
                f_idx = s - 1
                s2, s_lo = divmod(f_idx, 2)
                c, s2c = divmod(s2, CHUNK_SP)

                # prefetch next chunk at the start of each chunk
                if s2c == 0 and s_lo == 0 and c >= 1 and c + 1 < N_CHUNKS:
                    load_chunk(c + 1)

                ft = feat_tiles[c]
                e_s = ft[64 * s_lo : 64 * s_lo + T, s2c, :]

                ps = (psA, psB)[s % 2]
                pslot = (s // 2) % 8
                rhs = (
                    init_sb[:]
                    if s == 1
                    else state[:, (s - 2) % STATE_SLOTS, :]
                )
                nc.tensor.matmul(
                    ps[:, pslot, :], lhs_sb[:], rhs, start=True, stop=True
                )

                dst = state[:, (s - 1) % STATE_SLOTS, :]
                nc.vector.tensor_mul(dst, ps[:T, pslot, :], e_s)

                if s % NORM_K == 0 and s + NORM_LAG <= S:
                    j = s // NORM_K - 1
                    nc.vector.reciprocal_approx_fast(
                        r_buf[:, j, :], ps[SUM_ROW : SUM_ROW + 1, pslot, :]
                    )
                    nc.gpsimd.partition_broadcast(rb[:], r_buf[:, j, :], channels=T)
                    fa_idx = s + NORM_LAG - 1
                    fa2, fa_lo = divmod(fa_idx, 2)
                    fac, fa2c = divmod(fa2, CHUNK_SP)
                    e_app = feat_tiles[fac][64 * fa_lo : 64 * fa_lo + T, fa2c, :]
                    nc.vector.tensor_mul(e_app, e_app, rb[:])

                # dump every 16 steps
                if s % 16 == 0:
                    dc = s // 16 - 1
                    half = (dc % 2) * 16
                    nc.sync.dma_start(
                        out=dump[dc],
                        in_=state[:, half : half + 16, :],
                    )

            nc.sync.dma_start(out=r_out[:].unsqueeze(0), in_=r_buf[0:1, :, :])

    nc.finalize()
    return nc


def _get_nc():
    if "nc" not in _CACHED:
        _CACHED["nc"] = _build_nc()
    return _CACHED["nc"]


def kernel(features, transitions, seq_len):
    from concourse.bass_utils import run_bass_kernel_spmd

    features = np.asarray(features, dtype=np.float32)
    transitions = np.asarray(transitions, dtype=np.float32)
    seq_len_np = np.asarray(seq_len).astype(np.int64)

    nc = _get_nc()

    # host-side layout prep (sharding + transpose; no FLOPs beyond pad/transpose)
    trans_aug = np.full((T, M_COLS), -60000.0, dtype=np.float32)
    trans_aug[:, :T] = transitions.T  # lhsT[p, n] = transitions[n, p]
    trans_aug[:, SUM_ROW] = 0.0  # exp(0) = 1 -> ones column (sum row)
    init = np.zeros((T, BL), dtype=np.float32)
    init[START_TAG, :] = 1.0

    in_maps = []
    for c in range(N_CORES):
        fc = features[c * BL : (c + 1) * BL]  # [BL, S, T]
        # -> [128 = (s%2)*64+t (rows 48:64,112:128 pad), S//2, BL]
        ft = np.zeros((2, 64, S // 2, BL), dtype=np.float32)
        ft[:, :T] = fc.reshape(BL, S // 2, 2, T).transpose(2, 3, 1, 0)
        ft = np.ascontiguousarray(ft.reshape(128, S // 2, BL))
        in_maps.append(
            {"feats_t": ft, "trans_aug": trans_aug, "init_state": init}
        )

    global LAST_RESULT
    res = run_bass_kernel_spmd(
        nc, in_maps, core_ids=list(range(N_CORES)), trace=TRACE
    )
    LAST_RESULT = res

    # host epilogue ------------------------------------------------------
    ln2 = float(np.log(2.0))
    w_end = np.exp(transitions[END_TAG].astype(np.float64))  # [T]
    out = np.zeros(B, dtype=np.float64)
    for c in range(N_CORES):
        dump = res.results[c]["state_dump"].astype(np.float64)  # [128, T, 16, BL]
        r_arch = res.results[c]["r_out"].astype(np.float64)  # [128, BL]
        # log-scale applied up to and including step s:
        #   M_s = s * SCALE_SHIFT * ln2 - sum_{j: NORM_K*j <= s} ln r_j
        logr = np.log(r_arch)  # [N_EVENTS, BL]
        cum = np.cumsum(logr, axis=0)  # cum[j] = sum_{0..j}
        for bl in range(BL):
            L = int(seq_len_np[c * BL + bl])
            t_L = dump[(L - 1) // 16, :, (L - 1) % 16, bl]  # [T]
            M = L * SCALE_SHIFT * ln2
            # event j (1-based) applies its factor starting at step
            # NORM_K*j + NORM_LAG, i.e. included iff NORM_K*j + NORM_LAG <= L
            n_ev = (L - NORM_LAG) // NORM_K if L >= NORM_K + NORM_LAG else 0
            if n_ev > 0:
                M -= cum[n_ev - 1, bl]
            out[c * BL + bl] = np.log(np.dot(w_end, t_L)) + M
    return out.astype(np.float32)
